# revision 9
# baseline (speedup 1.0000x reference)
"""DiffWave forward pass on 8 Trainium2 NeuronCores (Bass/Tile).

Sharding: core c -> (batch b = c//2, sequence half h = c%2). Each core computes
its 8192-sample half over a window E = 8192 + 1024: the 1024-column halo is
refreshed twice (after layers 9 and 19, the dilation-cycle boundaries, where
the receptive field per 10-layer block is 1023) by a pairwise inter-core
exchange, instead of the communication-free 3072-column halo a full 30-layer
receptive field would need. Odd cores store their half TIME-MIRRORED (host
reverses their audio window and tap order of the dilated-conv weights, and
swaps the left/right edge-bias corrections), which makes the exchange fully
symmetric SPMD: every core sends local columns [E-2048, E-1024) forward,
and refreshes its margin [E-1024, E) with the partner's block read reversed.
The exchange itself is a pairwise ReduceScatter(add) through DRAM with the
send block duplicated into both slots; the receiver subtracts its own staged
copy to recover the partner's block (exact up to one f32 rounding).

Per-core layout: resident SBUF tensor xs[128, 1024+9216+1024] (float32r):
rows 0-63 = residual trunk x (deferred 1/sqrt(2) scaling folded into weights),
rows 64-127 = skip accumulator. Dilated conv = 3 accumulating float32r matmuls
(K=64, full rate at N=512) per 512-col slice reading shifted views of xs.
Gating runs as ONE packed ACT tanh per 1024-col chunk over all 128 partitions
(sigmoid(g) = (tanh(g/2)+1)/2 with the 0.5 folded into the gate-half weights
and biases), then a DMA partition-shift of the filter half and ONE in-place
DVE bf16 multiply P = tg*tf over the gate rows of sg. The (tg+1)*tf
expansion's "+tf" term is folded into the 1x1 conv instead: that conv runs as
a K=128 matmul whose lhsT carries the op weights duplicated on rows 0-63
(applied to P) and rows 64-127 (applied to tf) — matmul cost is K-independent
so this is free, and it removes the DVE +1 tensor_scalar op. The per-layer
conditioner bias (sum-of-taps dw_W @ cond + dw_b + deferred op-bias
corrections) is applied for free via the ACT bias operand, with edge-corrected
variants on the first/last dilation-width columns. In-layer in-place updates
are deferred by one chunk so neighbouring chunks read pre-update boundary
columns.
"""

import os
import sys

sys.path.insert(0, "/opt/trn_rl_repo")

import numpy as np

import concourse.bacc as bacc
import concourse.mybir as mybir
import concourse.tile as tile
from concourse.ap import AP

f32 = mybir.dt.float32
f32r = mybir.dt.float32r
bf16 = mybir.dt.bfloat16
AF = mybir.ActivationFunctionType
ALU = mybir.AluOpType

C = 64
L = 30
B = 4
T = 16384
MAX_STEPS = 200
OWN = T // 2          # 8192 owned samples per core
PAD = 512             # frozen zero pads (max dilation)
MARGIN = 1024         # halo refreshed at each 10-layer block boundary
E = OWN + MARGIN      # 9216 compute window
WBUF = PAD + E + PAD  # 10240
CH = 1024             # column chunk (2 PSUM banks)
NCH = E // CH         # 9
DILS = [2 ** (i % 10) for i in range(L)]
EXCH_AFTER = (9, 19)  # halo exchange after these layers

_CACHE = {}


def _rev_ap(t, ncols, width):
    """Reversed-column AP over tile t[128, ncols], columns [0, width) read
    backwards. Only valid for plain f32/bf16 tiles (f32r is layout-swizzled
    and mis-reads under negative stride)."""
    return AP(tensor=t[:].tensor, offset=width - 1, ap=[[ncols, 128], [-1, width]])


# --------------------------------------------------------------------------
# device program
# --------------------------------------------------------------------------
def _build_program(dbg=False):
    nc = bacc.Bacc(
        "TRN2",
        target_bir_lowering=False,
        debug=False,
        enable_asserts=False,
        num_devices=8,
    )

    dram = {}

    def din(name, shape, dtype):
        dram[name] = nc.dram_tensor(name, list(shape), dtype, kind="ExternalInput")
        return dram[name]

    din("aud", [1, E], f32r)
    din("w3", [C, L * 3 * 128], f32r)          # dilated conv lhsT per (l, tap)
    din("opw", [128, L * 128], bf16)           # 1x1 conv lhsT per l, rows duplicated
    din("wsum", [128, 15 * 128], f32)          # beff lhsT, layer pairs stacked
    din("bconst", [128, L], f32)               # beff constant term
    din("wtl", [128, 15 * 128], f32)           # left-edge lhsT (drops the -d tap)
    din("bcl", [128, L], f32)
    din("wtr", [128, 15 * 128], f32)           # right-edge lhsT (drops the +d tap)
    din("bcr", [128, L], f32)
    din("dpw", [128, 15 * 4 * 128], f32)       # cond lhsT, layer pairs x k-chunks
    din("dpb", [128, 15], f32)                 # cond bias, layer pairs stacked
    din("p1", [128, 512], f32)
    din("p1b", [128, 4], f32)
    din("p2", [128, 16 * 128], f32)
    din("p2b", [128, 4], f32)
    din("emb", [128, 1], f32)
    din("inw", [1, C], f32r)
    din("inb", [C, 1], f32)
    din("skw", [128, C], f32r)                 # rows 64-127 hold sk_W.T/sqrt(30)
    din("skb", [C, 1], f32)
    din("outw", [C, 1], bf16)
    din("outb", [1, 1], f32)
    din("zeros", [C, CH], f32r)
    o_d = nc.dram_tensor("o", [1, E], f32, kind="ExternalOutput")
    cc_in = [
        nc.dram_tensor(f"cc_in{i}", [256, MARGIN], f32, kind="Internal")
        for i in range(len(EXCH_AFTER))
    ]
    cc_out = [
        nc.dram_tensor(f"cc_out{i}", [128, MARGIN], f32, kind="Internal")
        for i in range(len(EXCH_AFTER))
    ]

    with tile.TileContext(nc) as tc:
        import contextlib

        ctx = contextlib.ExitStack()
        with ctx:
            const = ctx.enter_context(tc.tile_pool(name="const", bufs=1))
            sgp = ctx.enter_context(tc.tile_pool(name="sgp", bufs=3))
            sgfp = ctx.enter_context(tc.tile_pool(name="sgfp", bufs=3))
            hhp = ctx.enter_context(tc.tile_pool(name="hhp", bufs=2))
            otp = ctx.enter_context(tc.tile_pool(name="otp", bufs=1))
            exp = ctx.enter_context(tc.tile_pool(name="exp", bufs=2))
            dil_ps = ctx.enter_context(tc.tile_pool(name="dil_ps", bufs=2, space="PSUM"))
            op_ps = ctx.enter_context(tc.tile_pool(name="op_ps", bufs=2, space="PSUM"))

            # ---- resident state + weights ----
            xs = const.tile([128, WBUF], f32r)
            w3 = const.tile([C, L * 3 * 128], f32r)
            opw = const.tile([128, L * 128], bf16)
            wsum = const.tile([128, 15 * 128], f32)
            bconst = const.tile([128, L], f32)
            beff = const.tile([128, L], f32)
            wtl = const.tile([128, 15 * 128], f32)
            bcl = const.tile([128, L], f32)
            beffL = const.tile([128, L], f32)
            wtr = const.tile([128, 15 * 128], f32)
            bcr = const.tile([128, L], f32)
            beffR = const.tile([128, L], f32)
            inw = const.tile([1, C], f32r)
            inb = const.tile([C, 1], f32)
            skw = const.tile([128, C], f32r)
            skb = const.tile([C, 1], f32)
            outw = const.tile([C, 1], bf16)
            outb = const.tile([1, 1], f32)

            nc.sync.dma_start(w3[:], dram["w3"].ap())
            nc.sync.dma_start(opw[:], dram["opw"].ap())
            nc.sync.dma_start(wsum[:], dram["wsum"].ap())
            nc.sync.dma_start(bconst[:], dram["bconst"].ap())
            nc.sync.dma_start(wtl[:], dram["wtl"].ap())
            nc.sync.dma_start(bcl[:], dram["bcl"].ap())
            nc.sync.dma_start(wtr[:], dram["wtr"].ap())
            nc.sync.dma_start(bcr[:], dram["bcr"].ap())
            nc.sync.dma_start(inw[:], dram["inw"].ap())
            nc.sync.dma_start(inb[:], dram["inb"].ap())
            nc.sync.dma_start(skw[:], dram["skw"].ap())
            nc.sync.dma_start(skb[:], dram["skb"].ap())
            nc.sync.dma_start(outw[:], dram["outw"].ap())
            nc.sync.dma_start(outb[:], dram["outb"].ap())

            # ---- zero pads and skip accumulator (DMA: memset-f32r fails codegen) ----
            nc.sync.dma_start(xs[0:C, 0:PAD], dram["zeros"].ap()[:, 0:PAD])
            nc.sync.dma_start(xs[0:C, PAD + E : WBUF], dram["zeros"].ap()[:, 0:PAD])
            for c in range(WBUF // CH):
                nc.sync.dma_start(
                    xs[C:128, c * CH : (c + 1) * CH], dram["zeros"].ap()
                )

            with (
                tc.tile_pool(name="pre", bufs=1) as pre,
                tc.tile_pool(name="audp", bufs=2) as audp,
            ):
                # ---- diffusion embedding MLP + cond + beff (tiny, fp32) ----
                dpw = pre.tile([128, 15 * 4 * 128], f32)
                dpb = pre.tile([128, 15], f32)
                p1 = pre.tile([128, 512], f32)
                p1b = pre.tile([128, 4], f32)
                p2 = pre.tile([128, 16 * 128], f32)
                p2b = pre.tile([128, 4], f32)
                emb = pre.tile([128, 1], f32)
                t1 = pre.tile([128, 4], f32)
                t2 = pre.tile([128, 4], f32)
                cond = pre.tile([128, 15], f32)
                nc.sync.dma_start(dpw[:], dram["dpw"].ap())
                nc.sync.dma_start(dpb[:], dram["dpb"].ap())
                nc.sync.dma_start(p1[:], dram["p1"].ap())
                nc.sync.dma_start(p1b[:], dram["p1b"].ap())
                nc.sync.dma_start(p2[:], dram["p2"].ap())
                nc.sync.dma_start(p2b[:], dram["p2b"].ap())
                nc.sync.dma_start(emb[:], dram["emb"].ap())

                ps_t1 = dil_ps.tile([128, CH], f32, tag="dil")
                for i in range(4):
                    nc.tensor.matmul(
                        ps_t1[:, i : i + 1],
                        lhsT=p1[:, i * 128 : (i + 1) * 128],
                        rhs=emb[:, 0:1],
                        start=True,
                        stop=True,
                    )
                sgv1 = pre.tile([128, 4], f32)
                for i in range(4):
                    nc.scalar.activation(
                        sgv1[:, i : i + 1], ps_t1[:, i : i + 1], AF.Sigmoid,
                        bias=p1b[:, i : i + 1],
                    )
                for i in range(4):
                    # silu(v) = (ps + b) * sigmoid(ps + b)
                    nc.vector.scalar_tensor_tensor(
                        t1[:, i : i + 1], ps_t1[:, i : i + 1], p1b[:, i : i + 1],
                        sgv1[:, i : i + 1], ALU.add, ALU.mult,
                    )
                ps_t2 = op_ps.tile([128, CH], f32, tag="op")
                for i in range(4):
                    for j in range(4):
                        nc.tensor.matmul(
                            ps_t2[:, i : i + 1],
                            lhsT=p2[:, (i * 4 + j) * 128 : (i * 4 + j + 1) * 128],
                            rhs=t1[:, j : j + 1],
                            start=(j == 0),
                            stop=(j == 3),
                        )
                sgv2 = pre.tile([128, 4], f32)
                for i in range(4):
                    nc.scalar.activation(
                        sgv2[:, i : i + 1], ps_t2[:, i : i + 1], AF.Sigmoid,
                        bias=p2b[:, i : i + 1],
                    )
                for i in range(4):
                    nc.vector.scalar_tensor_tensor(
                        t2[:, i : i + 1], ps_t2[:, i : i + 1], p2b[:, i : i + 1],
                        sgv2[:, i : i + 1], ALU.add, ALU.mult,
                    )
                ps_cond = dil_ps.tile([128, CH], f32, tag="dil")
                for c in range(15):
                    for j in range(4):
                        nc.tensor.matmul(
                            ps_cond[:, c : c + 1],
                            lhsT=dpw[:, (c * 4 + j) * 128 : (c * 4 + j + 1) * 128],
                            rhs=t2[:, j : j + 1],
                            start=(j == 0),
                            stop=(j == 3),
                        )
                nc.vector.tensor_add(cond[:], ps_cond[:, 0:15], dpb[:])
                for wmat, bvec, bout in ((wsum, bconst, beff), (wtl, bcl, beffL), (wtr, bcr, beffR)):
                    ps_beff = op_ps.tile([128, CH], f32, tag="op", name="ps_beff")
                    for l in range(L):
                        c = l // 2
                        if l % 2 == 0:
                            nc.tensor.matmul(
                                ps_beff[:, l : l + 1],
                                lhsT=wmat[0:C, c * 128 : (c + 1) * 128],
                                rhs=cond[0:C, c : c + 1],
                                start=True,
                                stop=True,
                                tile_position=(0, 0),
                            )
                        else:
                            nc.tensor.matmul(
                                ps_beff[:, l : l + 1],
                                lhsT=wmat[C:128, c * 128 : (c + 1) * 128],
                                rhs=cond[C:128, c : c + 1],
                                start=True,
                                stop=True,
                                tile_position=(64, 0),
                            )
                    nc.vector.tensor_add(bout[:], ps_beff[:, 0:L], bvec[:])

                # ---- input conv: x0 = relu(in_W * audio + in_b) ----
                for c in range(NCH):
                    at = audp.tile([1, CH], f32r, tag="aud")
                    nc.sync.dma_start(at[:], dram["aud"].ap()[:, c * CH : (c + 1) * CH])
                    x0 = dil_ps.tile([128, CH], f32, tag="dil")
                    for s in (0, 512):
                        nc.tensor.matmul(
                            x0[0:C, s : s + 512],
                            lhsT=inw[:],
                            rhs=at[:, s : s + 512],
                            start=True,
                            stop=True,
                        )
                    nc.scalar.activation(
                        xs[0:C, PAD + c * CH : PAD + (c + 1) * CH],
                        x0[0:C, :],
                        AF.Relu,
                        bias=inb[:, 0:1],
                    )

            # ---- 30 residual layers ----
            for l in range(L):
                d = DILS[l]
                # chunk c+1's tap-0 matmuls read the last d columns of chunk c,
                # so chunk c's in-place update is emitted only after chunk c+1's
                # dilated-conv reads (one-chunk lag; Tile orders by program order)
                pend = None
                for c in range(NCH):
                    col = PAD + c * CH
                    dil = dil_ps.tile([128, CH], f32, tag="dil")
                    for k in range(3):
                        off = (k - 1) * d
                        for s in (0, 512):
                            nc.tensor.matmul(
                                dil[:, s : s + 512],
                                lhsT=w3[:, (l * 3 + k) * 128 : (l * 3 + k + 1) * 128],
                                rhs=xs[0:C, col + off + s : col + off + s + 512],
                                start=(k == 0),
                                stop=(k == 2),
                            )
                    sg = sgp.tile([128, CH], bf16, tag="sg")
                    # (column range, bias) pieces: sequence-edge columns use the
                    # tap-dropped bias (reference zero-pads x+cond, so the
                    # missing tap must not contribute cond/omega via the bias)
                    if c == 0:
                        pieces = [(0, d, beffL), (d, CH, beff)]
                    elif c == NCH - 1:
                        pieces = [(0, CH - d, beff), (CH - d, CH, beffR)]
                    else:
                        pieces = [(0, CH, beff)]
                    # single packed tanh: rows 0-63 hold tanh(g/2) (gate half
                    # pre-scaled 0.5 in weights; sigmoid(g) = (tanh(g/2)+1)/2),
                    # rows 64-127 hold tanh(f)
                    for lo, hi, bv in pieces:
                        nc.scalar.activation(
                            sg[:, lo:hi], dil[:, lo:hi], AF.Tanh,
                            bias=bv[:, l : l + 1],
                        )
                    sgf = sgfp.tile([C, CH], bf16, tag="sgf")
                    nc.sync.dma_start(sgf[:], sg[C:128, :])
                    # P = tg*tf in place over the gate rows; the op conv below
                    # contracts K=128 over [P; tf] with duplicated op weights,
                    # realizing (tg+1)*tf without a separate +1 op
                    nc.vector.tensor_mul(sg[0:C, :], sg[0:C, :], sgf[:])
                    if pend is not None:
                        pcol, pop = pend
                        nc.vector.tensor_add(
                            xs[:, pcol : pcol + CH], xs[:, pcol : pcol + CH], pop[:]
                        )
                    op = op_ps.tile([128, CH], f32, tag="op")
                    for s in (0, 512):
                        nc.tensor.matmul(
                            op[:, s : s + 512],
                            lhsT=opw[:, l * 128 : (l + 1) * 128],
                            rhs=sg[:, s : s + 512],
                            start=True,
                            stop=True,
                        )
                    pend = (col, op)
                pcol, pop = pend
                nc.vector.tensor_add(
                    xs[:, pcol : pcol + CH], xs[:, pcol : pcol + CH], pop[:]
                )

                # ---- halo exchange at dilation-block boundaries ----
                if l in EXCH_AFTER:
                    ex = EXCH_AFTER.index(l)
                    stage = exp.tile([128, MARGIN], f32, tag="stage")
                    nc.vector.tensor_copy(
                        stage[:], xs[:, PAD + E - 2 * MARGIN : PAD + E - MARGIN]
                    )
                    nc.sync.dma_start(cc_in[ex].ap()[0:128], stage[:])
                    nc.sync.dma_start(cc_in[ex].ap()[128:256], stage[:])
                    nc.gpsimd.collective_compute(
                        "ReduceScatter", ALU.add,
                        replica_groups=[[0, 1], [2, 3], [4, 5], [6, 7]],
                        ins=[cc_in[ex].ap()], outs=[cc_out[ex].ap()],
                    )
                    rt = exp.tile([128, MARGIN], f32, tag="rt")
                    nc.sync.dma_start(rt[:], cc_out[ex].ap())
                    # margin <- reverse(partner block) = reverse(rt - stage);
                    # both reads reversed, write forward into the f32r trunk
                    nc.vector.tensor_tensor(
                        xs[:, PAD + E - MARGIN : PAD + E],
                        _rev_ap(rt, MARGIN, MARGIN),
                        _rev_ap(stage, MARGIN, MARGIN),
                        ALU.subtract,
                    )

            # ---- tail: skip head, computed over the full window ----
            for c in range(NCH):
                col = PAD + c * CH
                hps = dil_ps.tile([128, CH], f32, tag="dil")
                for s in (0, 512):
                    nc.tensor.matmul(
                        hps[0:C, s : s + 512],
                        lhsT=skw[C:128, :],
                        rhs=xs[C:128, col + s : col + s + 512],
                        start=True,
                        stop=True,
                        tile_position=(64, 0),
                    )
                hh = hhp.tile([C, CH], bf16, tag="hh")
                nc.scalar.activation(hh[:], hps[0:C, :], AF.Relu, bias=skb[:, 0:1])
                ops2 = op_ps.tile([128, CH], f32, tag="op")
                for s in (0, 512):
                    nc.tensor.matmul(
                        ops2[0:1, s : s + 512],
                        lhsT=outw[:],
                        rhs=hh[:, s : s + 512],
                        start=True,
                        stop=True,
                    )
                ot = otp.tile([1, CH], f32, tag="ot")
                nc.vector.tensor_scalar_add(ot[:], ops2[0:1, :], outb[0:1, 0:1])
                nc.sync.dma_start(o_d.ap()[:, c * CH : (c + 1) * CH], ot[:])

    nc.compile()
    return nc


# --------------------------------------------------------------------------
# host-side weight folding
# --------------------------------------------------------------------------
def _emb_table():
    steps = np.arange(MAX_STEPS, dtype=np.float32)[:, None]
    dims = np.arange(64, dtype=np.float32)[None, :]
    t = steps * 10.0 ** (dims * 4.0 / 63.0)
    return np.concatenate([np.sin(t), np.cos(t)], axis=1).astype(np.float32)


def _prep_maps(inputs):
    f = lambda a: np.ascontiguousarray(np.asarray(a), dtype=np.float32)
    audio = f(inputs["audio"])          # [B,1,T]
    step = np.asarray(inputs["diffusion_step"]).astype(np.int64)  # [B]
    in_W, in_b = f(inputs["in_W"]), f(inputs["in_b"])
    p1_W, p1_b = f(inputs["p1_W"]), f(inputs["p1_b"])
    p2_W, p2_b = f(inputs["p2_W"]), f(inputs["p2_b"])
    dw_W, dw_b = f(inputs["dw_W"]), f(inputs["dw_b"])
    dp_W, dp_b = f(inputs["dp_W"]), f(inputs["dp_b"])
    op_W, op_b = f(inputs["op_W"]), f(inputs["op_b"])
    sk_W, sk_b = f(inputs["sk_W"]), f(inputs["sk_b"])
    out_W, out_b = f(inputs["out_W"]), f(inputs["out_b"])

    sc = np.float32(2.0) ** (-np.arange(L, dtype=np.float32) / 2)   # 2^(-l/2)
    scu = np.float32(2.0) ** (np.arange(L, dtype=np.float32) / 2)   # 2^(+l/2)

    # gate half computed as tanh(g/2): scale gate output channels by 0.5
    Sg = np.ones((128, 1), np.float32)
    Sg[0:C] = 0.5

    # dilated conv lhsT per parity: mirrored cores (h=1) run on the reversed
    # sequence, so their tap order flips (k -> 2-k)
    w3p = []
    for mir in (False, True):
        w3 = np.zeros((C, L * 3 * 128), np.float32)
        for l in range(L):
            for k in range(3):
                kk = 2 - k if mir else k
                w = dw_W[l, :, :, kk] * sc[l] * Sg        # [128(out), 64(in)]
                w3[:, (l * 3 + k) * 128 : (l * 3 + k + 1) * 128] = w.T
        w3p.append(w3)

    # 1x1 conv lhsT: input is (tanh(g/2)+1)*tanh(f) = 2*yg, so all cols * 0.5;
    # residual cols additionally * 2^(l/2) (deferred sqrt2). Rows duplicated:
    # the device computes opw.T @ (P + tf) with P = tg*tf on rows 0-63 and
    # tf on rows 64-127 of the K=128 rhs.
    opw = np.zeros((128, L * 128), np.float32)
    for l in range(L):
        w = op_W[l, :, :, 0] * 0.5                       # [128(out), 64(in)]
        w[0:C] *= scu[l]
        opw[0:C, l * 128 : (l + 1) * 128] = w.T
        opw[C:128, l * 128 : (l + 1) * 128] = w.T

    # beff = wsum @ cond + bconst;  wsum[l] = sum_k dw_W[l,:,:,k],
    # bconst[l] = dw_b[l] + 2^(-l/2) * wsum_raw[l] @ Omega_l,
    # Omega_l = sum_{j<l} 2^(j/2) * op_b[j,:64]
    # Edge variants drop the out-of-window tap; for mirrored cores the
    # local-left edge is the sequence END, so the L/R variants swap.
    wsum_raw = dw_W.sum(axis=3)                          # [L,128,64]
    wtl_raw = dw_W[:, :, :, 1:].sum(axis=3)              # drops tap 0 (the -d tap)
    wtr_raw = dw_W[:, :, :, :2].sum(axis=3)              # drops tap 2 (the +d tap)
    wsum = np.zeros((128, 15 * 128), np.float32)
    wtl = np.zeros((128, 15 * 128), np.float32)
    wtr = np.zeros((128, 15 * 128), np.float32)
    bconst = np.zeros((128, L), np.float32)
    bcl = np.zeros((128, L), np.float32)
    bcr = np.zeros((128, L), np.float32)
    omega = np.zeros(C, np.float32)
    for l in range(L):
        c = l // 2
        rows = slice(0, C) if l % 2 == 0 else slice(C, 128)
        cols = slice(c * 128, (c + 1) * 128)
        wsum[rows, cols] = (wsum_raw[l] * Sg).T
        wtl[rows, cols] = (wtl_raw[l] * Sg).T
        wtr[rows, cols] = (wtr_raw[l] * Sg).T
        bconst[:, l] = Sg[:, 0] * (dw_b[l] + sc[l] * (wsum_raw[l] @ omega))
        bcl[:, l] = Sg[:, 0] * (dw_b[l] + sc[l] * (wtl_raw[l] @ omega))
        bcr[:, l] = Sg[:, 0] * (dw_b[l] + sc[l] * (wtr_raw[l] @ omega))
        omega = omega + scu[l] * op_b[l, 0:C]

    # cond lhsT, layer pairs stacked on partitions, 4 k-chunks each
    dpw = np.zeros((128, 15 * 4 * 128), np.float32)
    dpb = np.zeros((128, 15), np.float32)
    for c in range(15):
        for j in range(4):
            blk = np.zeros((128, 128), np.float32)
            blk[:, 0:C] = dp_W[2 * c][:, j * 128 : (j + 1) * 128].T
            blk[:, C:128] = dp_W[2 * c + 1][:, j * 128 : (j + 1) * 128].T
            dpw[:, (c * 4 + j) * 128 : (c * 4 + j + 1) * 128] = blk
        dpb[0:C, c] = dp_b[2 * c]
        dpb[C:128, c] = dp_b[2 * c + 1]

    p1 = p1_W.T.copy()                                   # [128, 512]
    p1b = p1_b.reshape(4, 128).T.copy()
    p2 = np.zeros((128, 16 * 128), np.float32)
    p2T = p2_W.T
    for i in range(4):
        for j in range(4):
            p2[:, (i * 4 + j) * 128 : (i * 4 + j + 1) * 128] = p2T[
                j * 128 : (j + 1) * 128, i * 128 : (i + 1) * 128
            ]
    p2b = p2_b.reshape(4, 128).T.copy()

    # tail foldings
    opb_sk_sum = op_b[:, C:].sum(axis=0)                 # [64]
    skw = np.zeros((128, C), np.float32)
    skw[C:128] = (sk_W[:, :, 0] / np.sqrt(np.float32(L))).T
    skb = (sk_b + sk_W[:, :, 0] @ opb_sk_sum / np.sqrt(np.float32(L))).reshape(C, 1)
    outw = out_W[0, :, 0].reshape(C, 1)
    outb = out_b.reshape(1, 1)

    table = _emb_table()

    import ml_dtypes

    shared = {
        "opw": opw.astype(ml_dtypes.bfloat16),
        "wsum": wsum,
        "bconst": bconst,
        "dpw": dpw,
        "dpb": dpb,
        "p1": np.ascontiguousarray(p1),
        "p1b": np.ascontiguousarray(p1b),
        "p2": p2,
        "p2b": np.ascontiguousarray(p2b),
        "inw": in_W[:, 0, 0].reshape(1, C),
        "inb": in_b.reshape(C, 1),
        "skw": skw,
        "skb": skb,
        "outw": outw.astype(ml_dtypes.bfloat16),
        "outb": outb,
        "zeros": np.zeros((C, CH), np.float32),
    }

    in_maps = []
    for core in range(8):
        b, h = core // 2, core % 2
        m = dict(shared)
        m["w3"] = w3p[h]
        if h == 0:
            m["aud"] = np.ascontiguousarray(audio[b, 0, 0:E].reshape(1, E))
            m["wtl"], m["bcl"] = wtl, bcl
            m["wtr"], m["bcr"] = wtr, bcr
        else:
            # time-mirrored: reversed audio window; edge variants swap
            m["aud"] = np.ascontiguousarray(audio[b, 0, T - E : T][::-1].reshape(1, E))
            m["wtl"], m["bcl"] = wtr, bcr
            m["wtr"], m["bcr"] = wtl, bcl
        m["emb"] = np.ascontiguousarray(table[int(step[b])].reshape(128, 1))
        in_maps.append(m)
    return in_maps


def _get_nc():
    if "nc" not in _CACHE:
        _CACHE["nc"] = _build_program()
    return _CACHE["nc"]


def unshard(res_o):
    """res_o: list of 8 per-core 'o' arrays [1, E] -> full [B, 1, T]."""
    out = np.zeros((B, 1, T), np.float32)
    for b in range(B):
        out[b, 0, 0:OWN] = res_o[2 * b][0, 0:OWN]
        out[b, 0, OWN:T] = res_o[2 * b + 1][0, 0:OWN][::-1]
    return out


def kernel(**inputs) -> np.ndarray:
    from concourse.bass_utils import run_bass_kernel_spmd

    nc = _get_nc()
    in_maps = _prep_maps(inputs)
    res = run_bass_kernel_spmd(nc, in_maps, core_ids=list(range(8))).results
    return unshard([res[c]["o"] for c in range(8)])
